# revision 34
# baseline (speedup 1.0000x reference)
"""Causal self-attention + residual + LayerNorm — Trainium2, v6b.

Problem: B=4, S=2048, D=1024, H=16 heads (hd=64), fp32 in/out.
Sharding: zig-zag + pairwise K/V exchange. Core c -> batch c % 4,
query-group g = c // 4; g=0 owns query chunks 0 and 3, g=1 owns 1 and 2.
Each core computes only HALF the K/V projections (g=0: heads 0-7, g=1:
heads 8-15, all 4 chunks); halves are exchanged with one round of
pairwise HBM AllGathers ([[0,4],[1,5],[2,6],[3,7]]) whose transfer
overlaps Q-projection + attention on the own-half heads (own-first head
order per group). Two branch sections only (tile-pool slot rotation
cannot hand off across different If statements), with per-section PSUM
pools for the projection stream.

Other structure: causal clipping of masked k-tiles (matmul/exp moving
ranges start at column 128*band_idx), single [P,P] triangle mask,
pair-exp on Act, software-pipelined scores/ctx, denominator via ones
column in V, LN stats as ones-matmuls (1/D-ones stationary lands mu
pre-broadcast in PSUM), rstd broadcast via a 1-contract PE matmul,
LN(qt0) finish overlapped with out-proj(qt1), LN(qt1) as the only tail,
normalize split DVE/GpSimd in bf16 with Act doing the f32 widen.
"""
import sys

if "/opt/trn_rl_repo" not in sys.path:
    sys.path.insert(0, "/opt/trn_rl_repo")

import numpy as np
import ml_dtypes

B, S, D, H, HD = 4, 2048, 1024, 16, 64
P = 128
QT = 512
NQ = 1024
NKT = S // P                  # 16
DK = D // P                   # 8
NPLAIN = {0: (0, 12), 1: (4, 8)}   # group -> per-q-tile plain k-tiles
QCHUNK = {0: (0, 3), 1: (1, 2)}    # group -> global 512-query chunks owned
ESCALE = 0.125                # 1/sqrt(hd)
RG = [[0, 4], [1, 5], [2, 6], [3, 7]]

BB = np.dtype(ml_dtypes.bfloat16)

_cache = {}


def _build():
    import concourse.mybir as mybir
    import concourse.tile as tile
    from concourse import bacc
    from concourse.bass import ts
    from concourse.alu_op_type import AluOpType

    f32 = mybir.dt.float32
    bf16 = mybir.dt.bfloat16
    AF = mybir.ActivationFunctionType

    nc = bacc.Bacc("TRN2", target_bir_lowering=False, debug=False, num_devices=8)

    xbd = nc.dram_tensor("xb", [D, S], bf16, kind="ExternalInput").ap()
    wkbd = nc.dram_tensor("wkb", [D, D], bf16, kind="ExternalInput").ap()
    wqbd = nc.dram_tensor("wqb", [D, D], bf16, kind="ExternalInput").ap()
    wvbd = nc.dram_tensor("wvb", [D, D], bf16, kind="ExternalInput").ap()
    wobd = nc.dram_tensor("wob", [D, D], bf16, kind="ExternalInput").ap()
    maskbd = nc.dram_tensor("maskb", [P, P], bf16, kind="ExternalInput").ap()
    bqd = nc.dram_tensor("bqd", [D], f32, kind="ExternalInput").ap()
    bkd = nc.dram_tensor("bkd", [D], f32, kind="ExternalInput").ap()
    bod = nc.dram_tensor("bod", [D], f32, kind="ExternalInput").ap()
    gamd = nc.dram_tensor("gamd", [D], f32, kind="ExternalInput").ap()
    betd = nc.dram_tensor("betd", [D], f32, kind="ExternalInput").ap()
    yt = nc.dram_tensor("yt", [D, NQ], f32, kind="ExternalOutput").ap()

    xb_r = xbd.rearrange("(dk p) t -> p dk t", p=P)

    with tile.TileContext(nc) as tc:
        with tc.tile_pool(name="persist", bufs=1) as pers:
            kt = pers.tile([P, DK, S], bf16)            # K^T       32 KB/part
            v8 = pers.tile([P, NKT, H, HD + 1], bf16)   # V + ones  32.5 KB
            xt = pers.tile([P, DK, S], bf16)            # x^T       32 KB
            mkb = pers.tile([P, P], bf16)               # causal triangle mask
            bia = pers.tile([P, DK, 5], f32)            # bq bk bo gam bet
            ones128 = pers.tile([P, 1], bf16)
            onesD = pers.tile([P, P], bf16)             # 1/D: mu bcast-accum
            ones1 = pers.tile([1, P], bf16)             # rstd bcast stationary
            eps_t = pers.tile([1, 1], f32)
            nc.vector.memset(eps_t[:], 1e-5)
            nc.vector.memset(ones128[:], 1.0)
            nc.vector.memset(onesD[:], 1.0 / D)
            nc.vector.memset(ones1[:], 1.0)
            nc.vector.memset(v8[:, :, :, HD], 1.0)

            # startup-critical loads: x^T chunks 0/1 split fine across the
            # 3 DMA-capable rings so the first kproj can start early
            rings = (nc.sync, nc.scalar, nc.gpsimd, nc.gpsimd)
            for t in range(2):
                for piece in range(4):
                    sl = slice(2 * piece, 2 * piece + 2)
                    rings[piece].dma_start(
                        xt[:, sl, ts(t, QT)], xb_r[:, sl, ts(t, QT)])
            nc.gpsimd.dma_start(mkb[:], maskbd[:])
            for j, src in enumerate((bqd, bkd, bod, gamd, betd)):
                nc.gpsimd.dma_start(bia[:, :, j], src.rearrange("(f p) -> p f", p=P))

            def bq_(f): return bia[:, f, 0:1]
            def bk_(f): return bia[:, f, 1:2]
            def bo_(f): return bia[:, f, 2:3]
            def gam_(f): return bia[:, f, 3:4]
            def bet_(f): return bia[:, f, 4:5]

            with (
                tc.tile_pool(name="qtp", bufs=1) as qtp,
                tc.tile_pool(name="ctxp", bufs=2) as ctxp,
                tc.tile_pool(name="sep", bufs=2) as sep,
                tc.tile_pool(name="scr", bufs=1) as scr,
                tc.tile_pool(name="wqp", bufs=2) as wqp,
                tc.tile_pool(name="ep", bufs=1) as ep,
                tc.tile_pool(name="drp", bufs=1, space="DRAM") as drp,
            ):
                # DRAM bounce buffers for the pairwise K/V exchange
                k_i = drp.tile([P, 4, S], bf16, name="k_i")
                k_o = drp.tile([2, P, 4, S], bf16, name="k_o")
                v_i = drp.tile([P, NKT, 8, HD + 1], bf16, name="v_i")
                v_o = drp.tile([2, P, NKT, 8, HD + 1], bf16, name="v_o")

                pools = {}

                def kproj_group(t, floc, g, wk):
                    f = 4 * g + floc
                    ps = pools["pp"].tile([P, QT], f32, tag="pp")
                    for dk in range(DK):
                        nc.tensor.matmul(
                            ps[:], wk[:, dk, floc, :], xt[:, dk, ts(t, QT)],
                            start=(dk == 0), stop=(dk == DK - 1))
                    nc.vector.tensor_scalar_add(kt[:, f, ts(t, QT)], ps[:], bk_(f))

                def vproj_group(t, g, wv):
                    ps = pools["pp"].tile([P, QT], f32, tag="pp")
                    for dk in range(DK):
                        nc.tensor.matmul(
                            ps[:], xt[:, dk, ts(t, P)], wv[:, dk, :],
                            start=(dk == 0), stop=(dk == DK - 1))
                    nc.vector.tensor_copy(v8[:, t, 8 * g:8 * g + 8, 0:HD], ps[:])

                def qproj_group(qc, f, qtile):
                    wq = wqp.tile([P, DK, P], bf16, tag="wq")
                    nc.sync.dma_start(
                        wq[:], wqbd[:, ts(f, P)].rearrange("(dk p) c -> p dk c", p=P))
                    ps = pools["pp"].tile([P, QT], f32, tag="pp")
                    for dk in range(DK):
                        nc.tensor.matmul(
                            ps[:], wq[:, dk, :], xt[:, dk, ts(qc, QT)],
                            start=(dk == 0), stop=(dk == DK - 1))
                    nc.vector.tensor_scalar_add(qtile[:, f, :], ps[:], bq_(f))

                def attn_head(h, n_plain, qtile, ctx, aps):
                    nk = n_plain + 4
                    nu = nk // 2
                    hp, base = h // 2, HD * (h % 2)
                    cp = aps.tile([HD + 1, QT], f32, tag="cp", bufs=2)

                    # first valid query column of k-tile i (causal clip):
                    # band idx = i - n_plain, valid q >= 128*idx
                    def v0_(i):
                        return P * (i - n_plain) if i >= n_plain else 0

                    def scores(u):
                        sp = aps.tile([P, 2, QT], f32, tag="sp", bufs=2)
                        se = sep.tile([P, 2, QT], bf16, tag="se")
                        masked = 2 * u >= n_plain
                        for j in (0, 1):
                            i = 2 * u + j
                            v0 = v0_(i)
                            nc.tensor.matmul(
                                sp[:, j, v0:QT],
                                kt[base:base + HD, hp, ts(i, P)],
                                qtile[base:base + HD, hp, v0:QT],
                                start=True, stop=True)
                        if not masked:
                            nc.scalar.activation(
                                se[:], sp[:], AF.Exp, scale=ESCALE)
                        else:
                            for j in (0, 1):
                                v0 = v0_(2 * u + j)
                                nc.scalar.activation(
                                    se[:, j, v0:QT], sp[:, j, v0:QT],
                                    AF.Exp, scale=ESCALE)
                                nc.vector.tensor_mul(
                                    se[:, j, v0:v0 + P], se[:, j, v0:v0 + P],
                                    mkb[:])
                        return se

                    # software pipeline: scores(u+1) issue ahead of ctx(u)
                    # so the in-order PE never blocks on the exp feedback
                    se_cur = scores(0)
                    for u in range(nu):
                        se_nxt = scores(u + 1) if u + 1 < nu else None
                        for j in (0, 1):
                            i = 2 * u + j
                            v0 = v0_(i)
                            nc.tensor.matmul(
                                cp[:, v0:QT], v8[:, i, h, :],
                                se_cur[:, j, v0:QT],
                                start=(i == 0), stop=(i == nk - 1))
                        se_cur = se_nxt
                    den = scr.tile([1, QT], f32, tag="den")
                    nc.vector.tensor_copy(den[:], cp[HD:HD + 1, :])
                    rec = scr.tile([1, QT], f32, tag="rec")
                    rscr = scr.tile([1, QT], f32, tag="rscr")
                    nc.vector.reciprocal_approx_accurate(rec[:], den[:], rscr[:])
                    bc = scr.tile([HD, QT], f32, tag="bc")
                    nc.gpsimd.partition_broadcast(bc[:], rec[:])
                    po, ft = HD * (h % 2), h // 2
                    nc.vector.tensor_mul(ctx[po:po + HD, ft, :], cp[0:HD, :], bc[:])

                def out_group(qc, o, ctx, y, wob, bc_ps=None, ms_ps=None):
                    ps = pools["pp"].tile([P, QT], f32, tag="pp")
                    for dk in range(DK):
                        nc.tensor.matmul(
                            ps[:], wob[:, dk, o, :], ctx[:, dk, :],
                            start=(dk == 0), stop=(dk == DK - 1))
                    nc.vector.scalar_tensor_tensor(
                        y[:, o, :], ps[:], bo_(o), xt[:, o, ts(qc, QT)],
                        AluOpType.add, AluOpType.add)
                    if bc_ps is not None:
                        stats_piece(y, o, bc_ps, ms_ps)

                def stats_piece(y, o, bc_ps, ms_ps):
                    # LN stats of one o-block: the 1/D-ones stationary makes
                    # the mu accumulation land PRE-BROADCAST in [P, QT] PSUM
                    nc.tensor.matmul(
                        bc_ps[:, 0, :], onesD[:], y[:, o, :],
                        start=(o == 0), stop=(o == DK - 1))
                    ysq = ep.tile([P, QT], bf16, tag="ysq")
                    nc.vector.tensor_mul(ysq[:], y[:, o, :], y[:, o, :])
                    nc.tensor.matmul(
                        ms_ps[:], ones128[:], ysq[:],
                        start=(o == 0), stop=(o == DK - 1))

                def ln_chain(bc_ps, ms_ps):
                    """Stats -> broadcast bf16 (mu, rstd) rows in SBUF."""
                    mu_sb = ep.tile([1, QT], f32, tag="mu_sb")
                    nc.vector.tensor_copy(mu_sb[:], bc_ps[0:1, 0, :])
                    tmp = ep.tile([1, QT], f32, tag="stat_tmp", bufs=2)
                    nc.vector.tensor_mul(tmp[:], mu_sb[:], mu_sb[:])
                    ms = ep.tile([1, QT], f32, tag="ms_sb")
                    nc.vector.tensor_scalar_mul(ms[:], ms_ps[:], 1.0 / D)
                    nc.vector.tensor_sub(ms[:], ms[:], tmp[:])  # var
                    sd = ep.tile([1, QT], f32, tag="stat_tmp", bufs=2)
                    nc.scalar.activation(sd[:], ms[:], AF.Sqrt, bias=eps_t[:])
                    rstd = ep.tile([1, QT], f32, tag="ms_sb")
                    rsc = ep.tile([1, QT], f32, tag="stat_tmp", bufs=2)
                    nc.vector.reciprocal_approx_accurate(rstd[:], sd[:], rsc[:])
                    rs16 = ep.tile([1, QT], bf16, tag="rs16", bufs=2)
                    nc.vector.tensor_copy(rs16[:], rstd[:])
                    # broadcast rstd across partitions via a 1-contract matmul
                    nc.tensor.matmul(
                        bc_ps[:, 1, :], ones1[:], rs16[:],
                        start=True, stop=True)
                    st_bc = ep.tile([P, 2, QT], bf16, tag="st_bc", bufs=2)
                    nc.vector.tensor_copy(st_bc[:], bc_ps[:])
                    return st_bc

                def ln_po(st_bc, y, qt, o):
                    # one o-block of the LN normalize; bf16 SBUF = 2x DVE;
                    # GpSimd takes 2 of 8 (its tensor ops are ~3.5x slower)
                    eng = nc.gpsimd if o % 4 == 1 else nc.vector
                    mu_bc, rs_bc = st_bc[:, 0, :], st_bc[:, 1, :]
                    t1 = ep.tile([P, QT], bf16, tag="t1", bufs=2)
                    eng.tensor_sub(t1[:], y[:, o, :], mu_bc)
                    eng.tensor_mul(t1[:], t1[:], rs_bc)
                    eng.tensor_scalar(
                        t1[:], t1[:], gam_(o), bet_(o),
                        AluOpType.mult, AluOpType.add)
                    yo = ep.tile([P, QT], f32, tag="yo", bufs=2)
                    nc.scalar.activation(yo[:], t1[:], AF.Copy)
                    ring = nc.sync if o % 2 == 0 else nc.scalar
                    ring.dma_start(yt[ts(o, P), ts(qt, QT)], yo[:])

                def groupA(g, wk, wv):
                    """Own-half weights in; all own-half K/V projections;
                    store the halves for the exchange."""
                    fo = 4 * g
                    wk_r = wkbd[:, ts(g, QT)].rearrange(
                        "(dk p) (f c) -> p dk f c", p=P, c=P)
                    for fz in range(4):
                        eng = nc.scalar if fz % 2 else nc.sync
                        eng.dma_start(wk[:, :, fz, :], wk_r[:, :, fz, :])
                    wv_r = wvbd[:, ts(g, QT)].rearrange(
                        "(dk p) c -> p dk c", p=P)
                    for hz in range(2):
                        eng = nc.sync if hz else nc.scalar
                        eng.dma_start(
                            wv[:, :, ts(hz, 256)], wv_r[:, :, ts(hz, 256)])

                    for t in range(2):
                        for floc in range(4):
                            kproj_group(t, floc, g, wk)
                    # late x chunks: consumed by the t>=2 projections below
                    for t2 in (2, 3):
                        for piece in range(4):
                            sl = slice(2 * piece, 2 * piece + 2)
                            rings[piece].dma_start(
                                xt[:, sl, ts(t2, QT)], xb_r[:, sl, ts(t2, QT)])
                    for t in range(8):
                        vproj_group(t, g, wv)
                    for t in range(2, 4):
                        for floc in range(4):
                            kproj_group(t, floc, g, wk)
                    for t in range(8, NKT):
                        vproj_group(t, g, wv)

                    hwn = slice(8 * g, 8 * g + 8)
                    nc.sync.dma_start(k_i[:, :, 0:NQ], kt[:, fo:fo + 4, 0:NQ])
                    nc.scalar.dma_start(k_i[:, :, NQ:S], kt[:, fo:fo + 4, NQ:S])
                    nc.sync.dma_start(v_i[:, 0:8], v8[:, 0:8, hwn, :])
                    nc.scalar.dma_start(v_i[:, 8:NKT], v8[:, 8:NKT, hwn, :])

                def groupB(g, wob):
                    qc0, qc1 = QCHUNK[g]
                    pr, fp = 1 - g, 4 * (1 - g)
                    hpe = slice(8 * (1 - g), 8 * (1 - g) + 8)
                    own_first = list(range(8 * g, 8 * g + 8)) + \
                        list(range(8 * (1 - g), 8 * (1 - g) + 8))

                    qt0 = qtp.tile([P, DK, QT], bf16, tag="qtile")
                    ctx0 = ctxp.tile([P, DK, QT], bf16, tag="ctx")
                    with tc.tile_pool(name="aps0", bufs=1, space="PSUM") as aps:
                        # q-proj first so its wq stream is ahead of the
                        # cc-gated peer loads on the sync ring
                        for f in range(DK):
                            qproj_group(qc0, f, qt0)
                        nc.sync.dma_start(
                            kt[:, fp:fp + 4, 0:NQ], k_o[pr][:, :, 0:NQ])
                        nc.sync.dma_start(
                            v8[:, 0:8, hpe, :], v_o[pr][:, 0:8])
                        nc.sync.dma_start(
                            kt[:, fp:fp + 4, NQ:S], k_o[pr][:, :, NQ:S])
                        nc.sync.dma_start(
                            v8[:, 8:NKT, hpe, :], v_o[pr][:, 8:NKT])

                        for h in own_first:
                            attn_head(h, NPLAIN[g][0], qt0, ctx0, aps)

                        qt1 = qtp.tile([P, DK, QT], bf16, tag="qtile")
                        for f in range(DK):
                            qproj_group(qc1, f, qt1)
                        wo_r = wobd.rearrange(
                            "(dk p) (o c) -> p dk o c", p=P, c=P)
                        for oz in range(2):
                            nc.sync.dma_start(
                                wob[:, :, 4 * oz:4 * oz + 4, :],
                                wo_r[:, :, 4 * oz:4 * oz + 4, :])

                    # attention qt1 with out-proj(qt0) as PE filler
                    ctx1 = ctxp.tile([P, DK, QT], bf16, tag="ctx")
                    y0 = ep.tile([P, DK, QT], bf16, tag="y", bufs=2)
                    with tc.tile_pool(name="aps1", bufs=1, space="PSUM") as aps:
                        for hi, h in enumerate(own_first):
                            attn_head(h, NPLAIN[g][1], qt1, ctx1, aps)
                            if hi < DK:
                                out_group(qc0, hi, ctx0, y0, wob)

                    # final: out-proj(qt1) + both LN stats interleaved;
                    # LN(qt0) finish overlaps the out-proj stream
                    y1 = ep.tile([P, DK, QT], bf16, tag="y", bufs=2)
                    with tc.tile_pool(name="stp", bufs=1, space="PSUM") as stp:
                        bc0 = stp.tile([P, 2, QT], f32, tag="bc0")
                        ms0 = stp.tile([1, QT], f32, tag="ms0")
                        bc1 = stp.tile([P, 2, QT], f32, tag="bc1")
                        ms1 = stp.tile([1, QT], f32, tag="ms1")
                        st_bc0 = None
                        for o in range(DK):
                            out_group(qc1, o, ctx1, y1, wob, bc1, ms1)
                            if o < 4:
                                stats_piece(y0, 2 * o, bc0, ms0)
                                stats_piece(y0, 2 * o + 1, bc0, ms0)
                            else:
                                ln_po(st_bc0, y0, 0, 2 * (o - 4))
                                ln_po(st_bc0, y0, 0, 2 * (o - 4) + 1)
                            if o == 3:
                                st_bc0 = ln_chain(bc0, ms0)
                        st_bc1 = ln_chain(bc1, ms1)
                        for o in range(DK):
                            ln_po(st_bc1, y1, 1, o)

                pid = nc.partition_id()
                with (
                    tc.tile_pool(name="wkv", bufs=1) as wkvp,
                    tc.tile_pool(name="ppA", bufs=2, space="PSUM") as ppA,
                ):
                    pools["pp"] = ppA
                    wk = wkvp.tile([P, DK, 4, P], bf16)
                    wv = wkvp.tile([P, DK, QT], bf16)
                    with tc.If(pid < 4) as cA:
                        groupA(0, wk, wv)
                    with cA.Else():
                        groupA(1, wk, wv)

                for i_, o_ in ((k_i, k_o), (v_i, v_o)):
                    nc.gpsimd.collective_compute(
                        "AllGather", mybir.AluOpType.bypass,
                        replica_groups=RG,
                        ins=[i_.opt()], outs=[o_.opt()])

                with (
                    tc.tile_pool(name="wo2", bufs=1) as wo2p,
                    tc.tile_pool(name="ppB", bufs=2, space="PSUM") as ppB,
                ):
                    pools["pp"] = ppB
                    wob = wo2p.tile([P, DK, DK, P], bf16)
                    with tc.If(pid < 4) as cB:
                        groupB(0, wob)
                    with cB.Else():
                        groupB(1, wob)
    nc.compile()
    return nc


def _get_nc():
    if "nc" not in _cache:
        _cache["nc"] = _build()
    return _cache["nc"]


def _prep(x, in_proj_w, in_proj_b, out_w, out_b, gamma, beta):
    x = np.asarray(x, np.float32)
    wt = np.ascontiguousarray(np.asarray(in_proj_w, np.float32).T)
    wot = np.ascontiguousarray(np.asarray(out_w, np.float32).T)
    bqkv = np.asarray(in_proj_b, np.float32)
    bo = np.asarray(out_b, np.float32)
    gam = np.asarray(gamma, np.float32)
    bet = np.asarray(beta, np.float32)

    wqb = np.ascontiguousarray(wt[:, 0:D].astype(BB))
    wkb = np.ascontiguousarray(wt[:, D:2 * D].astype(BB))
    wvb = np.ascontiguousarray(wt[:, 2 * D:3 * D].astype(BB))
    wob = wot.astype(BB)
    bo_eff = bo + wot.T @ bqkv[2 * D:3 * D]

    # single [P, P] upper-right triangle mask: after causal clipping every
    # masked k-tile's partial region is the same local triangle l >= p
    p = np.arange(P)[:, None]
    l = np.arange(P)[None, :]
    maskb = np.ascontiguousarray((l >= p).astype(BB))

    qcols = {
        0: np.r_[0:QT, 3 * QT:4 * QT],
        1: np.r_[QT:3 * QT],
    }
    in_maps = []
    for cidx in range(8):
        b = cidx % 4
        xbt = np.ascontiguousarray(x[b].T.astype(BB))
        in_maps.append({
            "xb": xbt,
            "wqb": wqb, "wkb": wkb, "wvb": wvb, "wob": wob,
            "maskb": maskb,
            "bqd": bqkv[0:D], "bkd": bqkv[D:2 * D], "bod": bo_eff,
            "gamd": gam, "betd": bet,
        })
    return in_maps, qcols


def _run(in_maps, trace=False, **kw):
    from concourse.bass_utils import run_bass_kernel_spmd

    return run_bass_kernel_spmd(_get_nc(), in_maps, list(range(8)), trace=trace, **kw)


def kernel(x, in_proj_w, in_proj_b, out_w, out_b, gamma, beta):
    in_maps, qcols = _prep(x, in_proj_w, in_proj_b, out_w, out_b, gamma, beta)
    res = _run(in_maps)
    out = np.empty((B, S, D), np.float32)
    for c in range(8):
        out[c % 4, qcols[c // 4]] = res.results[c]["yt"].T
    return out
